# revision 3
# baseline (speedup 1.0000x reference)
"""Trainium2 Bass kernel for nn_Block_13709535609166 (dense transformer block).

B=8, T=1024, D=1024, H=16, HD=64, FF=4096. Data-parallel over batch: one
batch element per NeuronCore (8 cores), no collectives. All matmuls bf16
with fp32 PSUM accumulation; residual stream and LN/softmax arithmetic fp32.

Self-contained: hardcodes shapes/sharding; only needs numpy/ml_dtypes and
the concourse (Bass) stack available in the container image.
"""
import numpy as np
import ml_dtypes

import concourse.bass as bass
import concourse.mybir as mybir
import concourse.tile as tile
from concourse import bacc
from concourse.masks import make_identity

BF16 = mybir.dt.bfloat16
F32 = mybir.dt.float32
AF = mybir.ActivationFunctionType
ALU = mybir.AluOpType
GELU_AF = AF.Gelu  # swapped to Tanh for CoreSim (Gelu not implemented in sim)

B, T, D, H = 8, 1024, 1024, 16
HD = D // H  # 64
FF = 4 * D
TC = T // 128   # 8 token chunks
DC = D // 128   # 8 feature chunks
FC = FF // 128  # 32 ff chunks
NT = T // 512   # 2 free-dim chunks of 512 tokens
NF = D // 512   # 2 free-dim chunks of 512 features


def build_block_kernel(nc):
    """Emit the full transformer block for one batch element."""
    dram = {}
    for name, shape, dt in [
        ("x", [T, D], F32),
        ("wq", [D, D], BF16), ("wk", [D, D], BF16), ("wv", [D, D], BF16),
        ("bq", [D], F32), ("bk", [D], F32), ("bv", [D], F32),
        ("wo", [D, D], BF16), ("bo", [D], F32),
        ("w1", [FC, 128, DC, 128], BF16), ("b1", [FF], F32),
        ("w2", [FC, NF, 128, 512], BF16), ("b2", [D], F32),
        ("ln1_g", [D], F32), ("ln1_b", [D], F32),
        ("ln2_g", [D], F32), ("ln2_b", [D], F32),
    ]:
        dram[name] = nc.dram_tensor(name, shape, dt, kind="ExternalInput").ap()
    out_d = nc.dram_tensor("out", [T, D], F32, kind="ExternalOutput").ap()
    out_r = out_d.rearrange("(m p) d -> p m d", p=128)

    with tile.TileContext(nc) as tc:
        _emit(nc, tc, dram, out_r)
    return nc


def _emit(nc, tc, dram, out_r):
    from contextlib import ExitStack

    with ExitStack() as ctx:
        consts = ctx.enter_context(tc.tile_pool(name="consts", bufs=1))
        resid = ctx.enter_context(tc.tile_pool(name="resid", bufs=1))
        work = ctx.enter_context(tc.tile_pool(name="work", bufs=3))

        # ---- constants ----
        ident = consts.tile([128, 128], BF16)
        make_identity(nc, ident)
        ones64 = consts.tile([128, 64], BF16)
        nc.vector.memset(ones64, 1.0)
        eps_t = consts.tile([128, 1], F32)
        nc.vector.memset(eps_t, 1e-5)

        col = {}
        for name in ["ln1_g", "ln1_b", "ln2_g", "ln2_b", "bq", "bk"]:
            col[name] = consts.tile([128, DC], F32, name=f"c_{name}")
            nc.sync.dma_start(col[name], dram[name].rearrange("(o p) -> p o", p=128))
        col["b1"] = consts.tile([128, FC], F32, name="c_b1")
        nc.sync.dma_start(col["b1"], dram["b1"].rearrange("(o p) -> p o", p=128))
        bcast = {}
        for name in ["bv", "bo", "b2"]:
            bcast[name] = consts.tile([128, D], F32, name=f"bc_{name}")
            nc.gpsimd.dma_start(bcast[name], dram[name][None, :].partition_broadcast(128))

        # ---- residual stream (token-major fp32, updated in place) ----
        x_sb = resid.tile([128, TC, D], F32)
        nc.sync.dma_start(x_sb, dram["x"].rearrange("(m p) d -> p m d", p=128))

        def layer_norm_to_fm(x_src, g_col, b_col, h_fm, tr_pool):
            """Token-major LN on x_src -> transposed bf16 feature-major h_fm."""
            for m in range(TC):
                stats = work.tile([128, 2, 6], F32, name="stats")
                nc.vector.bn_stats(out=stats[:, 0, :], in_=x_src[:, m, 0:512])
                nc.vector.bn_stats(out=stats[:, 1, :], in_=x_src[:, m, 512:1024])
                mv = work.tile([128, 2], F32, name="mv")
                nc.vector.bn_aggr(out=mv, in_=stats)
                std = work.tile([128, 1], F32, name="std")
                nc.scalar.activation(out=std, in_=mv[:, 1:2], func=AF.Sqrt,
                                     bias=eps_t, scale=1.0)
                rstd = work.tile([128, 1], F32, name="rstd")
                nc.vector.reciprocal(rstd, std)
                h_norm = work.tile([128, D], BF16, name="h_norm")
                nc.vector.tensor_scalar(
                    out=h_norm, in0=x_src[:, m, :], scalar1=mv[:, 0:1], scalar2=rstd,
                    op0=ALU.subtract, op1=ALU.mult)
                for ko in range(DC):
                    tr_ps = tr_pool.tile([128, 128], BF16, name="tr")
                    nc.tensor.transpose(tr_ps, h_norm[:, 128 * ko:128 * ko + 128], ident)
                    nc.vector.tensor_scalar(
                        out=h_fm[:, ko, 128 * m:128 * m + 128], in0=tr_ps,
                        scalar1=g_col[:, ko:ko + 1], scalar2=b_col[:, ko:ko + 1],
                        op0=ALU.mult, op1=ALU.add)

        # Long-lived pools, opened early to respect LIFO pool discipline.
        # Stack (bottom->top): h2 | ctx | qkv | <phase-local pools>
        st = ExitStack()
        p_h2 = st.enter_context(tc.tile_pool(name="h2_sb", bufs=1))
        h2_fm = p_h2.tile([128, DC, T], BF16, name="h2_fm")
        st_ctx = ExitStack()
        p_ctx = st_ctx.enter_context(tc.tile_pool(name="ctx_sb", bufs=1))
        ctx_fm = p_ctx.tile([128, DC, T], BF16, name="ctx_fm")
        st_qkv = ExitStack()
        p_qkv = st_qkv.enter_context(tc.tile_pool(name="qkv_sb", bufs=1))
        q_fm = p_qkv.tile([128, DC, T], BF16, name="q_fm")
        k_fm = p_qkv.tile([128, DC, T], BF16, name="k_fm")
        v_sb = p_qkv.tile([128, TC, D], BF16, name="v_sb")

        # ================= Phase 1: LN1 + QKV =================
        st_p1 = ExitStack()
        p1 = st_p1.enter_context(tc.tile_pool(name="p1_sb", bufs=1))
        tr_pool = st_p1.enter_context(tc.tile_pool(name="p1_tr", bufs=4, space="PSUM"))
        mm_pool = st_p1.enter_context(tc.tile_pool(name="p1_mm", bufs=4, space="PSUM"))

        h_fm = p1.tile([128, DC, T], BF16, name="h_fm")
        layer_norm_to_fm(x_sb, col["ln1_g"], col["ln1_b"], h_fm, tr_pool)

        with tc.tile_pool(name="p1_wqk", bufs=1) as p_wqk:
            wq_sb = p_wqk.tile([128, DC, D], BF16, name="wq_sb")
            wk_sb = p_wqk.tile([128, DC, D], BF16, name="wk_sb")
            nc.sync.dma_start(wq_sb, dram["wq"].rearrange("(o p) q -> p o q", p=128))
            nc.sync.dma_start(wk_sb, dram["wk"].rearrange("(o p) q -> p o q", p=128))
            # Q, K: feature-major [qfeat, t]; lhsT = W chunk, rhs = h_fm
            for dst, wt, bias in [(q_fm, wq_sb, col["bq"]), (k_fm, wk_sb, col["bk"])]:
                for m in range(DC):
                    ps = [mm_pool.tile([128, 512], F32, name="mm") for _ in range(NT)]
                    for ko in range(DC):
                        for tn in range(NT):
                            nc.tensor.matmul(
                                ps[tn], lhsT=wt[:, ko, 128 * m:128 * m + 128],
                                rhs=h_fm[:, ko, 512 * tn:512 * tn + 512],
                                start=(ko == 0), stop=(ko == DC - 1))
                    for tn in range(NT):
                        nc.vector.tensor_scalar(
                            out=dst[:, m, 512 * tn:512 * tn + 512], in0=ps[tn],
                            scalar1=bias[:, m:m + 1], scalar2=None, op0=ALU.add)
        with tc.tile_pool(name="p1_wv", bufs=1) as p_wv:
            wv_sb = p_wv.tile([128, DC, D], BF16, name="wv_sb")
            nc.sync.dma_start(wv_sb, dram["wv"].rearrange("(o p) q -> p o q", p=128))
            # V: token-major [t, vfeat]; lhsT = h_fm chunk, rhs = W chunk
            for m in range(TC):
                ps = [mm_pool.tile([128, 512], F32, name="mm") for _ in range(NF)]
                for ko in range(DC):
                    for fn in range(NF):
                        nc.tensor.matmul(
                            ps[fn], lhsT=h_fm[:, ko, 128 * m:128 * m + 128],
                            rhs=wv_sb[:, ko, 512 * fn:512 * fn + 512],
                            start=(ko == 0), stop=(ko == DC - 1))
                for fn in range(NF):
                    nc.vector.tensor_add(
                        out=v_sb[:, m, 512 * fn:512 * fn + 512], in0=ps[fn],
                        in1=bcast["bv"][:, 512 * fn:512 * fn + 512])
        st_p1.close()

        # ================= Phase 2: attention =================
        with (
            tc.tile_pool(name="p2_e", bufs=6) as e_pool,
            tc.tile_pool(name="p2_sc", bufs=4, space="PSUM") as sc_pool,
            tc.tile_pool(name="p2_hold", bufs=1, space="PSUM") as hold_pool,
        ):
            for pc in range(DC):  # head pair = q/k feature chunk
                for tn in range(NT):
                    se_ps = [hold_pool.tile([64, 512], F32, name=f"se{h}") for h in range(2)]
                    cx_ps = [hold_pool.tile([64, 512], F32, name=f"cx{h}") for h in range(2)]
                    for sm in range(TC):
                        for h in range(2):
                            lo = 64 * h
                            sc = sc_pool.tile([128, 512], F32, name="sc")
                            nc.tensor.matmul(
                                sc, lhsT=k_fm[lo:lo + 64, pc, 128 * sm:128 * sm + 128],
                                rhs=q_fm[lo:lo + 64, pc, 512 * tn:512 * tn + 512],
                                start=True, stop=True)
                            e = e_pool.tile([128, 512], BF16, name="e")
                            nc.scalar.activation(out=e, in_=sc, func=AF.Exp, scale=0.125)
                            nc.tensor.matmul(
                                se_ps[h], lhsT=ones64, rhs=e,
                                start=(sm == 0), stop=(sm == TC - 1))
                            nc.tensor.matmul(
                                cx_ps[h],
                                lhsT=v_sb[:, sm, 128 * pc + lo:128 * pc + lo + 64],
                                rhs=e, start=(sm == 0), stop=(sm == TC - 1))
                    for h in range(2):
                        lo = 64 * h
                        recip = work.tile([64, 512], F32, name="recip")
                        nc.vector.reciprocal(recip, se_ps[h])
                        nc.vector.tensor_mul(
                            out=ctx_fm[lo:lo + 64, pc, 512 * tn:512 * tn + 512],
                            in0=cx_ps[h], in1=recip)
        st_qkv.close()

        # ============ Phase 3: out-proj + residual + LN2 ============
        with (
            tc.tile_pool(name="p3_sb", bufs=1) as p3,
            tc.tile_pool(name="p3_mm", bufs=4, space="PSUM") as mm3_pool,
            tc.tile_pool(name="p3_tr", bufs=4, space="PSUM") as tr3_pool,
        ):
            wo_sb = p3.tile([128, DC, D], BF16, name="wo_sb")
            nc.sync.dma_start(wo_sb, dram["wo"].rearrange("(o p) q -> p o q", p=128))
            for m in range(TC):
                ps = [mm3_pool.tile([128, 512], F32, name="mm") for _ in range(NF)]
                for ko in range(DC):
                    for fn in range(NF):
                        nc.tensor.matmul(
                            ps[fn], lhsT=ctx_fm[:, ko, 128 * m:128 * m + 128],
                            rhs=wo_sb[:, ko, 512 * fn:512 * fn + 512],
                            start=(ko == 0), stop=(ko == DC - 1))
                for fn in range(NF):
                    sl = slice(512 * fn, 512 * fn + 512)
                    # x += attn_out + bo  (in-place residual update)
                    nc.vector.tensor_add(out=x_sb[:, m, sl], in0=ps[fn], in1=x_sb[:, m, sl])
                    nc.vector.tensor_add(out=x_sb[:, m, sl], in0=x_sb[:, m, sl],
                                         in1=bcast["bo"][:, sl])
            layer_norm_to_fm(x_sb, col["ln2_g"], col["ln2_b"], h2_fm, tr3_pool)
        st_ctx.close()

        # ================= Phase 4: FFN =================
        st_g1 = ExitStack()
        p_g1 = st_g1.enter_context(tc.tile_pool(name="g1_sb", bufs=1))
        g1_fm = p_g1.tile([128, FC, T], BF16, name="g1_fm")

        with (
            tc.tile_pool(name="p4_w1", bufs=4) as w1_pool,
            tc.tile_pool(name="p4_mm1", bufs=4, space="PSUM") as mm4_pool,
        ):
            for mf in range(FC):
                w1_t = w1_pool.tile([128, DC, 128], BF16, name="w1t")
                nc.sync.dma_start(w1_t, dram["w1"][mf])
                ps = [mm4_pool.tile([128, 512], F32, name="mm") for _ in range(NT)]
                for ko in range(DC):
                    for tn in range(NT):
                        nc.tensor.matmul(
                            ps[tn], lhsT=w1_t[:, ko, :],
                            rhs=h2_fm[:, ko, 512 * tn:512 * tn + 512],
                            start=(ko == 0), stop=(ko == DC - 1))
                for tn in range(NT):
                    nc.scalar.activation(
                        out=g1_fm[:, mf, 512 * tn:512 * tn + 512], in_=ps[tn],
                        func=GELU_AF, bias=col["b1"][:, mf:mf + 1], scale=1.0)

        with (
            tc.tile_pool(name="p4_w2", bufs=4) as w2_pool,
            tc.tile_pool(name="p4_out", bufs=4) as out_pool,
            tc.tile_pool(name="p4_mm2", bufs=8, space="PSUM") as mm4b_pool,
        ):
            MH = TC // 2  # token chunks per half
            for th in range(2):
                ps = [[mm4b_pool.tile([128, 512], F32, name="mm") for _ in range(NF)]
                      for _ in range(MH)]
                for ko in range(FC):
                    w2_t = w2_pool.tile([128, NF, 512], BF16, name="w2t")
                    nc.sync.dma_start(w2_t, dram["w2"][ko].rearrange("f p c -> p f c"))
                    for m4 in range(MH):
                        m = MH * th + m4
                        for fn in range(NF):
                            nc.tensor.matmul(
                                ps[m4][fn], lhsT=g1_fm[:, ko, 128 * m:128 * m + 128],
                                rhs=w2_t[:, fn, :],
                                start=(ko == 0), stop=(ko == FC - 1))
                for m4 in range(MH):
                    m = MH * th + m4
                    for fn in range(NF):
                        sl = slice(512 * fn, 512 * fn + 512)
                        o = out_pool.tile([128, 512], F32, name="o")
                        nc.vector.tensor_add(out=o, in0=ps[m4][fn], in1=x_sb[:, m, sl])
                        nc.vector.tensor_add(out=o, in0=o, in1=bcast["b2"][:, sl])
                        nc.sync.dma_start(out_r[:, m, sl], o)
        st_g1.close()
        st.close()


_BUILT = {}


def _get_built():
    if "nc" not in _BUILT:
        nc = bacc.Bacc("TRN2", target_bir_lowering=False, debug=False,
                       enable_asserts=False, num_devices=8)
        build_block_kernel(nc)
        nc.compile()
        _BUILT["nc"] = nc
    return _BUILT["nc"]


def prep_inputs(inputs):
    """Host-side reshape/cast of the full (unsharded) inputs."""
    bf = ml_dtypes.bfloat16
    f32 = np.float32

    def flat_heads(w):  # [H, D, HD] -> [D, H*HD]
        return np.ascontiguousarray(np.transpose(np.asarray(w, f32), (1, 0, 2))
                                    .reshape(D, D)).astype(bf)

    common = {
        "wq": flat_heads(inputs["Wq"]), "wk": flat_heads(inputs["Wk"]),
        "wv": flat_heads(inputs["Wv"]),
        "bq": np.asarray(inputs["bq"], f32).reshape(D).copy(),
        "bk": np.asarray(inputs["bk"], f32).reshape(D).copy(),
        "bv": np.asarray(inputs["bv"], f32).reshape(D).copy(),
        "wo": np.ascontiguousarray(np.asarray(inputs["Wo"], f32)).astype(bf),
        "bo": np.asarray(inputs["bo"], f32).copy(),
        "w1": np.ascontiguousarray(
            np.asarray(inputs["W1"], f32).reshape(DC, 128, FC, 128)
            .transpose(2, 1, 0, 3)).astype(bf),
        "b1": np.asarray(inputs["b1"], f32).copy(),
        "w2": np.ascontiguousarray(
            np.asarray(inputs["W2"], f32).reshape(FC, 128, NF, 512)
            .transpose(0, 2, 1, 3)).astype(bf),
        "b2": np.asarray(inputs["b2"], f32).copy(),
        "ln1_g": np.asarray(inputs["ln1_g"], f32).copy(),
        "ln1_b": np.asarray(inputs["ln1_b"], f32).copy(),
        "ln2_g": np.asarray(inputs["ln2_g"], f32).copy(),
        "ln2_b": np.asarray(inputs["ln2_b"], f32).copy(),
    }
    x = np.asarray(inputs["x"], f32)
    in_maps = [dict(common, x=np.ascontiguousarray(x[b])) for b in range(B)]
    return in_maps


def run_on_hw(inputs, trace=False):
    from concourse import bass_utils
    nc = _get_built()
    in_maps = prep_inputs(inputs)
    res = bass_utils.run_bass_kernel_spmd(nc, in_maps, core_ids=list(range(B)),
                                          trace=trace)
    out = np.stack([res.results[b]["out"] for b in range(B)], axis=0)
    return out, res


def kernel(**inputs):
    out, _ = run_on_hw(inputs, trace=False)
    return out


def make_test_inputs(seed=0):
    rng = np.random.default_rng(seed)
    return {
        "x": rng.standard_normal((B, T, D)).astype(np.float32),
        "ln1_g": np.ones(D, np.float32), "ln1_b": np.zeros(D, np.float32),
        "ln2_g": np.ones(D, np.float32), "ln2_b": np.zeros(D, np.float32),
        "Wq": (rng.standard_normal((H, D, HD)) * 0.02).astype(np.float32),
        "bq": np.zeros((H, HD), np.float32),
        "Wk": (rng.standard_normal((H, D, HD)) * 0.02).astype(np.float32),
        "bk": np.zeros((H, HD), np.float32),
        "Wv": (rng.standard_normal((H, D, HD)) * 0.02).astype(np.float32),
        "bv": np.zeros((H, HD), np.float32),
        "Wo": (rng.standard_normal((D, D)) * 0.02).astype(np.float32),
        "bo": np.zeros(D, np.float32),
        "W1": (rng.standard_normal((D, FF)) * 0.02).astype(np.float32),
        "b1": np.zeros(FF, np.float32),
        "W2": (rng.standard_normal((FF, D)) * 0.02).astype(np.float32),
        "b2": np.zeros(D, np.float32),
    }


def np_ref_single(ins, xb, gelu="erf"):
    """float64 numpy reference for one batch element."""
    from scipy.special import erf

    def ln(v):
        mu = v.mean(-1, keepdims=True)
        var = ((v - mu) ** 2).mean(-1, keepdims=True)
        return (v - mu) / np.sqrt(var + 1e-5)

    Wq = np.transpose(ins["Wq"], (1, 0, 2)).reshape(D, D)
    Wk = np.transpose(ins["Wk"], (1, 0, 2)).reshape(D, D)
    Wv = np.transpose(ins["Wv"], (1, 0, 2)).reshape(D, D)
    h = ln(xb) * ins["ln1_g"] + ins["ln1_b"]
    q = h @ Wq + ins["bq"].reshape(-1)
    k = h @ Wk + ins["bk"].reshape(-1)
    v = h @ Wv + ins["bv"].reshape(-1)
    ctxs = []
    for hh in range(H):
        sl = slice(hh * HD, hh * HD + HD)
        sc = q[:, sl] @ k[:, sl].T / np.sqrt(HD)
        a = np.exp(sc - sc.max(-1, keepdims=True))
        a /= a.sum(-1, keepdims=True)
        ctxs.append(a @ v[:, sl])
    ctx = np.concatenate(ctxs, -1)
    xb = xb + ctx @ ins["Wo"] + ins["bo"]
    h2 = ln(xb) * ins["ln2_g"] + ins["ln2_b"]
    ff1 = h2 @ ins["W1"] + ins["b1"]
    if gelu == "tanh":
        g = np.tanh(ff1)
    else:
        g = 0.5 * ff1 * (1 + erf(ff1 / np.sqrt(2)))
    return xb + g @ ins["W2"] + ins["b2"]


if __name__ == "__main__":
    import sys
    mode = sys.argv[1] if len(sys.argv) > 1 else "sim"
    ins = make_test_inputs()
    if mode == "sim":
        import kernel as _self
        globals()["GELU_AF"] = AF.Tanh
        nc = bacc.Bacc("TRN2", target_bir_lowering=False, debug=False,
                       enable_asserts=False)
        build_block_kernel(nc)
        in_map = prep_inputs(ins)[0]
        from concourse.bass_interp import CoreSim
        sim = CoreSim(nc, trace=False)
        for name, arr in in_map.items():
            sim.tensor(name)[:] = arr
        sim.simulate()
        got = np.array(sim.tensor("out"))
        ref = np_ref_single(ins, ins["x"][0].astype(np.float64), gelu="tanh")
        rel = np.linalg.norm(got - ref) / np.linalg.norm(ref)
        print(f"sim maxabs={np.abs(got - ref).max():.5f} relnorm={rel:.6f}")
    else:
        out, res = run_on_hw(ins, trace=False)
        ref = np_ref_single(ins, ins["x"][0].astype(np.float64))
        rel = np.linalg.norm(out[0] - ref) / np.linalg.norm(ref)
        print(f"hw b0 maxabs={np.abs(out[0] - ref).max():.5f} relnorm={rel:.6f}")
